# revision 41
# baseline (speedup 1.0000x reference)
"""Multi-head attention block kernel for Trainium2, sharded over 8 NeuronCores.

Sharding: batch (4) x head-group (2 groups of 8 heads) -> 8 cores.
Each core computes, for one batch b and one half of the heads:
  qh/kh/vh projections (columns of w_q/w_k/w_v for its heads),
  causal attention for its 8 heads, and a partial output projection
  (rows of w_o^T for its heads).  Host sums the two partial outputs per
  batch and transposes back.

On-chip layout is feature-major ("transposed"): activations live as
[feature, seq] so every matmul contraction dim is on partitions and no
on-chip transposes are needed.  Host pre-transposes q/k/v and the
weight slices, and post-transposes the output.

Key HW-calibrated design points (measured on TRN2):
- A matmul with K=64 contraction runs ~2.4x slower than K=128 (510 vs
  216 ns for N=512).  Score matmuls contract over head_dim=64, so the
  score stationary kh is stored zero-padded per head (khz[128,h,S]:
  the other head's 64 partitions are zero).  qh stays dense (both
  heads); the zero rows of the stationary null out the other head.
- The Activation engine (exp) costs free_size*0.83ns + ~185ns fixed,
  so exps for a head pair are fused into one [128, 2, 512] instruction
  reading two PSUM banks at once.
- Causal N-trim: diagonal k-tiles only compute columns >= kt*128
  within the q-chunk (saves ~15% of score/exp/attn@V work); only the
  [128,128] diagonal block needs a triangular mask multiply.
- The attention inner loop is ACT-bound (exp), so the PE idles ~1us per
  k-tile step.  All projection (phase 1) and output-projection (phase
  3) matmuls are chopped into small "filler" thunks and emitted inside
  those bubbles.  The schedule wraps around the timing loop: during
  q-chunk qc the fillers are phase-1 for chunk qc+1 (chunk 0 of the
  next iteration when qc=3) and phase-3 for chunk qc-1 (chunk 3 of the
  previous iteration when qc=0), with a prologue/epilogue outside the
  loop for the first/last iteration.
- Normalization denominators come free from a ones column appended to
  each V tile.  The accumulator PSUM banks are released quickly (one ACT
  copy for the values, one DVE copy moving the denominator row to
  partition 0 in f32); 1/denom uses the single-instruction
  reciprocal_approx_fast (the 5-op exact reciprocal alone used to cost
  ~57us of wall time); the partition broadcast (Pool engine) and the
  final multiplies are deferred into the filler queue.

Matmuls run in bf16 (fp32 matmul is 4x slower on TRN2); accumulation is
fp32 in PSUM.
"""

import sys

sys.path.insert(0, "/opt/trn_rl_repo")

import numpy as np
import ml_dtypes

import concourse.bacc as bacc
import concourse.mybir as mybir
import concourse.tile as tile
from concourse import bass_utils

B = 4
S = 2048
E = 1024
HEADS = 16
D = 64
H = 8            # heads per core
F = H * D        # 512 local head features
P = 128
ET = E // P      # 8 e-tiles
FT = F // P      # 4 f-tiles (head pairs)
ST = S // P      # 16 s-tiles
QC = 512         # q-chunk width
NQC = S // QC    # 4 q-chunks
KT_PER_QC = QC // P  # 4 k-tiles per q-chunk

BF16 = mybir.dt.bfloat16
F32 = mybir.dt.float32
NPBF16 = ml_dtypes.bfloat16

EXP = mybir.ActivationFunctionType.Exp
COPY = mybir.ActivationFunctionType.Copy


def build_nc(causal: bool, niter: int | None = None, phases=(1, 2, 3),
             no_norm=False, no_exp=False, trim=True, interleave=True,
             sc_bufs=2, at_bufs=5, grp_bufs=2, xt_evict="act", mm_per_fill=2,
             av_lag=3, staggered=True, out_bf16=True, spread_dma=True,
             norm_mode=3, stream_bufs=1, tri_pool=False, smalls_bufs=2):
    """Build the per-core Bass program.  If niter is given, wrap the body in a
    For_i timing loop (used by test.py to measure HW time)."""
    nc = bacc.Bacc("TRN2", target_bir_lowering=False, debug=False,
                   enable_asserts=True, num_devices=8)

    qT = nc.dram_tensor("qT", [E, S], BF16, kind="ExternalInput").ap()
    kT = nc.dram_tensor("kT", [E, S], BF16, kind="ExternalInput").ap()
    vT = nc.dram_tensor("vT", [E, S], BF16, kind="ExternalInput").ap()
    wqT = nc.dram_tensor("wqT", [E, F], BF16, kind="ExternalInput").ap()
    wkT = nc.dram_tensor("wkT", [E, F], BF16, kind="ExternalInput").ap()
    wvT = nc.dram_tensor("wvT", [E, F], BF16, kind="ExternalInput").ap()
    woT = nc.dram_tensor("woT", [F, E], BF16, kind="ExternalInput").ap()
    stair = nc.dram_tensor("stair", [P, 2 * QC], BF16, kind="ExternalInput").ap()
    if not causal:
        maskT = nc.dram_tensor("maskT", [S, S], BF16, kind="ExternalInput").ap()
    outT = nc.dram_tensor("outT", [E, S], BF16 if out_bf16 else F32,
                          kind="ExternalOutput").ap()

    qT3 = qT.rearrange("(o p) s -> p o s", p=P)
    kT3 = kT.rearrange("(o p) s -> p o s", p=P)
    vT3 = vT.rearrange("(o p) s -> p o s", p=P)
    outT3 = outT.rearrange("(o p) s -> p o s", p=P)
    if not causal:
        maskT3 = maskT.rearrange("(o p) s -> p o s", p=P)

    run1 = 1 in phases
    run2 = 2 in phases
    run3 = 3 in phases
    if not causal:
        # general-mask fallback streams [P, ST, QC] mask chunks; shrink the
        # attention pools so everything fits in SBUF
        at_bufs = min(at_bufs, 4)
        grp_bufs = grp_bufs  # unchanged

    with tile.TileContext(nc) as tc:
        import contextlib
        with contextlib.ExitStack() as ctx:
            persist = ctx.enter_context(tc.tile_pool(name="persist", bufs=1))
            streams = ctx.enter_context(
                tc.tile_pool(name="streams", bufs=stream_bufs if causal else 1))
            attnp = ctx.enter_context(tc.tile_pool(name="attnp", bufs=at_bufs))
            smalls = ctx.enter_context(tc.tile_pool(name="smalls", bufs=smalls_bufs))
            outp = ctx.enter_context(tc.tile_pool(name="outp", bufs=3))
            if not causal:
                maskp = ctx.enter_context(tc.tile_pool(name="maskp", bufs=1))
            ps_sc = ctx.enter_context(
                tc.tile_pool(name="ps_sc", bufs=sc_bufs, space="PSUM"))
            ps_xt = ctx.enter_context(
                tc.tile_pool(name="ps_xt", bufs=1, space="PSUM"))
            ps_grp = ctx.enter_context(
                tc.tile_pool(name="ps_grp", bufs=grp_bufs, space="PSUM"))

            # Weights + constants: loaded once, outside the timing loop.
            wq_sb = persist.tile([P, ET, F], BF16, tag="wq")
            wk_sb = persist.tile([P, ET, F], BF16, tag="wk")
            wv_sb = persist.tile([P, ET, F], BF16, tag="wv")
            wo_sb = persist.tile([P, FT, E], BF16, tag="wo")
            stair_sb = persist.tile([P, P], BF16, tag="stair")
            nc.sync.dma_start(wv_sb[:], wvT.rearrange("(o p) f -> p o f", p=P))
            nc.sync.dma_start(wq_sb[:], wqT.rearrange("(o p) f -> p o f", p=P))
            nc.sync.dma_start(wk_sb[:], wkT.rearrange("(o p) f -> p o f", p=P))
            # tri[i, j] = (j >= i), the relevant slice of the staircase
            nc.sync.dma_start(stair_sb[:], stair[:, QC:QC + P])
            nc.sync.dma_start(wo_sb[:], woT.rearrange("(o p) e -> p o e", p=P))
            tri = stair_sb[:]

            # Persistent activations (bf16).
            qh_sb = persist.tile([P, FT, S], BF16, tag="qh")      # [f, ft, s]
            khz_sb = persist.tile([P, H, S], BF16, tag="khz")     # zero-padded per head
            vh_sb = persist.tile([P, ST, H, D + 1], BF16, tag="vh")  # ones col at d=D
            xts_sb = persist.tile([P, FT, S], BF16, tag="xts")

            # One-time initialization (never rewritten inside the loop):
            # zero halves of khz, ones column of vh, and the xts chunk read
            # by the wrapped-around phase-3 of the (nonexistent) iteration -1.
            for h in range(H):
                lo, hi = (64, 128) if h % 2 == 0 else (0, 64)
                nc.vector.memset(khz_sb[lo:hi, h, :], 0.0)
            nc.vector.memset(vh_sb[:, :, :, D:D + 1], 1.0)
            if run3:
                nc.vector.memset(xts_sb[:, :, (NQC - 1) * QC:], 0.0)

            def body(prologue, epilogue):
                if not run1:
                    nc.vector.memset(qh_sb[:, :, 0:1], 0.5)
                    for h in range(H):
                        lo, hi = (0, 64) if h % 2 == 0 else (64, 128)
                        nc.vector.memset(khz_sb[lo:hi, h, 0:1], 0.5)
                    nc.vector.memset(vh_sb[:, :, :, 0:1], 0.5)
                if not run2 and run3:
                    nc.vector.memset(xts_sb[:, :, 0:1], 0.5)

                # ---------- phase-1 pieces (per seq-chunk sc) ----------
                def dma_chunk(sc):
                    xq = streams.tile([P, ET, QC], BF16, tag="xcq")
                    xk = streams.tile([P, ET, QC], BF16, tag="xck")
                    xv = streams.tile([P, ET, QC], BF16, tag="xcv")
                    eng_q = nc.scalar if spread_dma else nc.sync
                    eng_k = nc.gpsimd if spread_dma else nc.sync
                    eng_q.dma_start(xq[:], qT3[:, :, sc * QC:(sc + 1) * QC])
                    eng_k.dma_start(xk[:], kT3[:, :, sc * QC:(sc + 1) * QC])
                    nc.sync.dma_start(xv[:], vT3[:, :, sc * QC:(sc + 1) * QC])
                    return (xq, xk, xv)

                def grp_thunks(name, n_mm, mk_mm, finish):
                    """Chop one PSUM accumulation group into filler thunks."""
                    st = {}
                    thunks = []
                    for i0 in range(0, n_mm, mm_per_fill):
                        i1 = min(n_mm, i0 + mm_per_fill)
                        def t(i0=i0, i1=i1):
                            if "g" not in st:
                                st["g"] = ps_grp.tile([P, QC], F32, tag="g",
                                                      name=name)
                            for i in range(i0, i1):
                                mk_mm(st["g"], i)
                            if i1 == n_mm:
                                finish(st["g"])
                        thunks.append(t)
                    return thunks

                def p1_thunks(sc, xcs):
                    xq, xk, xv = xcs
                    thunks = []
                    for si in range(4):
                        st_ = 4 * sc + si

                        def vmm(g, et, si=si):
                            nc.tensor.matmul(
                                g[:], xv[:, et, si * P:(si + 1) * P],
                                wv_sb[:, et, :],
                                start=(et == 0), stop=(et == ET - 1))

                        def vfin(g, st_=st_):
                            nc.vector.tensor_copy(
                                vh_sb[:, st_, :, 0:D],
                                g[:].rearrange("p (h d) -> p h d", h=H))
                        thunks += grp_thunks(f"v{si}", ET, vmm, vfin)
                    for ft in range(FT):
                        def qmm(g, et, ft=ft):
                            nc.tensor.matmul(
                                g[:], wq_sb[:, et, ft * P:(ft + 1) * P],
                                xq[:, et, :],
                                start=(et == 0), stop=(et == ET - 1))

                        def qfin(g, ft=ft):
                            nc.vector.tensor_copy(
                                qh_sb[:, ft, sc * QC:(sc + 1) * QC], g[:])
                        thunks += grp_thunks(f"q{ft}", ET, qmm, qfin)
                    for ft in range(FT):
                        def kmm(g, et, ft=ft):
                            nc.tensor.matmul(
                                g[:], wk_sb[:, et, ft * P:(ft + 1) * P],
                                xk[:, et, :],
                                start=(et == 0), stop=(et == ET - 1))

                        def kfin(g, ft=ft):
                            nc.vector.tensor_copy(
                                khz_sb[0:64, 2 * ft, sc * QC:(sc + 1) * QC],
                                g[0:64, :])
                            nc.vector.tensor_copy(
                                khz_sb[64:128, 2 * ft + 1, sc * QC:(sc + 1) * QC],
                                g[64:128, :])
                        thunks += grp_thunks(f"k{ft}", ET, kmm, kfin)
                    return thunks

                def p3_thunks(qc):
                    thunks = []
                    for jt in range(ET):
                        def omm(g, ft, jt=jt):
                            nc.tensor.matmul(
                                g[:], wo_sb[:, ft, jt * P:(jt + 1) * P],
                                xts_sb[:, ft, qc * QC:(qc + 1) * QC],
                                start=(ft == 0), stop=(ft == FT - 1))

                        def ofin(g, jt=jt):
                            ot = outp.tile([P, QC], BF16 if out_bf16 else F32,
                                           tag="ot")
                            nc.vector.tensor_copy(ot[:], g[:])
                            eng_o = nc.gpsimd if spread_dma else nc.sync
                            eng_o.dma_start(
                                outT3[:, jt, qc * QC:(qc + 1) * QC], ot[:])
                        thunks += grp_thunks(f"o{jt}", FT, omm, ofin)
                    return thunks

                # ---------- phase 2: attention for (qc, head pair hp) ----------
                def attention(qc, hp, mc, fill):
                    h0, h1 = 2 * hp, 2 * hp + 1
                    ktm = (qc + 1) * KT_PER_QC if causal else ST
                    xt = ps_xt.tile([D + 1, 2, QC], F32, tag="xt", name="xt")
                    ats = {}

                    def emit_av(kt):
                        off = max(0, kt * P - qc * QC) if (causal and trim) else 0
                        at = ats.pop(kt)
                        for a, h in ((0, h0), (1, h1)):
                            nc.tensor.matmul(
                                xt[:, a, off:],
                                vh_sb[:, kt, h, :],
                                at[:, a, off:],
                                start=(kt == 0), stop=(kt == ktm - 1))

                    for kt in range(ktm):
                        off = max(0, kt * P - qc * QC) if (causal and trim) else 0
                        scp = ps_sc.tile([P, 2, QC], F32, tag="sc", name="scp")
                        for a, h in ((0, h0), (1, h1)):
                            nc.tensor.matmul(
                                scp[:, a, off:],
                                khz_sb[:, h, kt * P:(kt + 1) * P],
                                qh_sb[:, hp, qc * QC + off:(qc + 1) * QC],
                                start=True, stop=True)
                        at = attnp.tile([P, 2, QC], BF16, tag="at")
                        if no_exp:
                            nc.vector.tensor_copy(at[:, :, off:], scp[:, :, off:])
                        else:
                            nc.scalar.activation(at[:, :, off:], scp[:, :, off:],
                                                 EXP, scale=0.125)
                        if causal:
                            if kt >= qc * KT_PER_QC:
                                doff = kt * P - qc * QC
                                tri_eng = nc.gpsimd if tri_pool else nc.vector
                                for a in range(2):
                                    tri_eng.tensor_mul(
                                        at[:, a, doff:doff + P],
                                        at[:, a, doff:doff + P], tri)
                        else:
                            for a in range(2):
                                nc.vector.tensor_mul(
                                    at[:, a, :], at[:, a, :], mc[:, kt, :])
                        ats[kt] = at
                        fill()
                        # attn@V lags the exp stream so the PE never waits on
                        # the Activation engine's latency
                        if kt >= av_lag:
                            emit_av(kt - av_lag)
                    for kt in range(max(0, ktm - av_lag), ktm):
                        emit_av(kt)
                    # Evict accumulator + normalize from SBUF:
                    # xts[0:D] = xt[0:D] / xt[D]  (denominator from ones col).
                    # One wide ACT copy releases both PSUM banks; the
                    # broadcast + multiplies are deferred into the filler
                    # queue so they run inside the next head pair's ACT-bound
                    # bubbles instead of stalling the engines here.
                    if no_norm:
                        for a, h in ((0, h0), (1, h1)):
                            fo = (h % 2) * D
                            nc.vector.tensor_copy(
                                xts_sb[fo:fo + D, hp, qc * QC:(qc + 1) * QC],
                                xt[0:D, a, :])
                        return []
                    xtw = attnp.tile([D, 2, QC], BF16, tag="xtw")
                    recip = smalls.tile([1, 2, QC], F32, tag="recip")
                    with nc.allow_low_precision(reason="bf16 attn normalize"):
                        if xt_evict == "act":
                            nc.scalar.activation(xtw[:], xt[0:D, :, :], COPY)
                        else:
                            nc.vector.tensor_copy(xtw[:], xt[0:D, :, :])
                    if norm_mode >= 1:
                        # Denominator row to SBUF f32 at partition 0 (regular
                        # DVE copy handles the base-partition remap), then one
                        # single-instruction approximate reciprocal (custom
                        # DVE ops need base partition 0 + contiguous SBUF).
                        drow = smalls.tile([1, 2, QC], F32, tag="drow")
                        nc.vector.tensor_copy(drow[:], xt[D:D + 1, :, :])
                        nc.vector.reciprocal_approx_fast(
                            recip[:].rearrange("p a b -> p (a b)"),
                            drow[:].rearrange("p a b -> p (a b)"))
                    if norm_mode < 1:
                        for a, h in ((0, h0), (1, h1)):
                            fo = (h % 2) * D
                            nc.vector.tensor_copy(
                                xts_sb[fo:fo + D, hp, qc * QC:(qc + 1) * QC],
                                xtw[:, a, :])
                        return []

                    def nfin():
                        rb = smalls.tile([D, 2, QC], F32, tag="rb")
                        if norm_mode >= 2:
                            nc.gpsimd.partition_broadcast(
                                rb[:].rearrange("p a b -> p (a b)"),
                                recip[0:1, :, :].rearrange("p a b -> p (a b)"))
                        with nc.allow_low_precision(reason="bf16 attn normalize"):
                            for a, h in ((0, h0), (1, h1)):
                                fo = (h % 2) * D
                                if norm_mode >= 3:
                                    nc.vector.tensor_mul(
                                        xts_sb[fo:fo + D, hp,
                                               qc * QC:(qc + 1) * QC],
                                        xtw[:, a, :], rb[:, a, :])
                                else:
                                    nc.vector.tensor_copy(
                                        xts_sb[fo:fo + D, hp,
                                               qc * QC:(qc + 1) * QC],
                                        xtw[:, a, :])
                    return [nfin]

                # ---------- emission schedule ----------
                class Filler:
                    def __init__(self):
                        self.q = []
                        self.budget = 1

                    def push(self, thunks):
                        self.q.extend(thunks)

                    def set_rate(self, steps_left):
                        self.budget = max(1, -(-len(self.q) // max(1, steps_left)))

                    def fill(self):
                        for t in self.q[:self.budget]:
                            t()
                        del self.q[:self.budget]

                    def drain(self):
                        for t in self.q:
                            t()
                        self.q.clear()

                fl = Filler()

                if prologue and run1:
                    # first iteration's chunk-0 projections
                    xcs0 = dma_chunk(0)
                    for t in p1_thunks(0, xcs0):
                        t()

                single_shot = prologue and epilogue
                for qc in range(NQC):
                    if not causal and run2:
                        mc = maskp.tile([P, ST, QC], BF16, tag="mc")
                        nc.sync.dma_start(
                            mc[:], maskT3[:, :, qc * QC:(qc + 1) * QC])
                    else:
                        mc = None
                    if run1 and not (single_shot and qc == NQC - 1):
                        nxt = (qc + 1) % NQC
                        fl.push(p1_thunks(nxt, dma_chunk(nxt)))
                    if run3 and not (single_shot and qc == 0):
                        fl.push(p3_thunks((qc - 1) % NQC))
                    if run2:
                        steps = (qc + 1) * KT_PER_QC if causal else ST
                        for hp in range(FT):
                            if interleave:
                                fl.set_rate(steps * (FT - hp))
                                finish = attention(qc, hp, mc, fl.fill)
                                fl.push(finish)
                            else:
                                finish = attention(qc, hp, mc, lambda: None)
                                for t in finish:
                                    t()
                        fl.drain()
                    else:
                        fl.drain()

                if epilogue and run3:
                    for t in p3_thunks(NQC - 1):
                        t()

            if niter is None:
                body(True, True)
            else:
                body(True, False)  # prologue iteration's phase-1 chunk 0
                # Note: the first body() above only emits the *prologue* p1;
                # the timed loop below contains the steady-state schedule.
                with tc.For_i(0, niter, 1, staggered_reset=staggered):
                    body(False, False)
                body(False, True)  # epilogue: last iteration's final p3

    nc.compile()
    return nc


def _host_prep(q, k, v, mask, w_q, w_k, w_v, w_o):
    """Shard + transpose inputs on the host.  Returns (in_maps, causal)."""
    tril = np.tril(np.ones((S, S), dtype=mask.dtype))
    causal = all(np.array_equal(np.asarray(mask[b, 0]), tril) for b in range(B))

    stair = (np.arange(2 * QC)[None, :] >= (np.arange(P)[:, None] + QC))
    stair = stair.astype(NPBF16)

    w_q = np.asarray(w_q, dtype=np.float32)
    w_k = np.asarray(w_k, dtype=np.float32)
    w_v = np.asarray(w_v, dtype=np.float32)
    w_o = np.asarray(w_o, dtype=np.float32)

    in_maps = []
    for core in range(8):
        b, g = divmod(core, 2)
        rows = slice(g * F, (g + 1) * F)
        m = {
            "qT": np.ascontiguousarray(np.asarray(q[b], np.float32).T).astype(NPBF16),
            "kT": np.ascontiguousarray(np.asarray(k[b], np.float32).T).astype(NPBF16),
            "vT": np.ascontiguousarray(np.asarray(v[b], np.float32).T).astype(NPBF16),
            "wqT": np.ascontiguousarray(w_q[rows, :].T).astype(NPBF16),
            "wkT": np.ascontiguousarray(w_k[rows, :].T).astype(NPBF16),
            "wvT": np.ascontiguousarray(w_v[rows, :].T).astype(NPBF16),
            "woT": np.ascontiguousarray(w_o[:, rows].T).astype(NPBF16),
            "stair": stair,
        }
        if not causal:
            m["maskT"] = np.ascontiguousarray(
                np.asarray(mask[b, 0], np.float32).T).astype(NPBF16)
        in_maps.append(m)
    return in_maps, causal


_NC_CACHE: dict = {}


def kernel(q, k, v, mask, w_q, w_k, w_v, w_o):
    in_maps, causal = _host_prep(q, k, v, mask, w_q, w_k, w_v, w_o)
    nc = _NC_CACHE.get(causal)
    if nc is None:
        nc = build_nc(causal)
        _NC_CACHE[causal] = nc
    res = bass_utils.run_bass_kernel_spmd(nc, in_maps, core_ids=list(range(8)))
    out = np.empty((B, S, E), dtype=np.float32)
    for b in range(B):
        out[b] = (res.results[2 * b]["outT"].astype(np.float32)
                  + res.results[2 * b + 1]["outT"].astype(np.float32)).T
    return out
